# revision 47
# baseline (speedup 1.0000x reference)
"""Trainium2 Bass kernel for nn_LinearLatentKernel_84834194031187.

Computes, for x:[B,S,D], W_qkv:[3D,D], W_gate:[D,D] (fp32):
    qkv = x @ W_qkv.T + b_qkv ; q,k,v = split(qkv)
    kv_state = cumsum(k*v, axis=seq)
    out = q * kv_state * sigmoid(x @ W_gate.T + b_gate)

Sharding: 8 cores = (batch b in 0..3) x (channel half h in 0..1); each core
computes out[b, :, h*512:(h+1)*512].

Layout: everything is computed TRANSPOSED, [channel, seq], so that
  - the projection matmuls take host-pretransposed x^T [D,S] directly as the
    moving operand (no PE transposes at all): ps[c,s] = wt[d,c]^T @ x^T[d,s]
  - the cumsum along seq becomes a FREE-dim prefix scan on the Vector engine
    (tensor_tensor_scan, chained across 512-col chunks via initial=prev[:,-1:])
so the Tensor engine runs ONLY the 1024 projection matmuls (8 kt x 4 types x
4 groups x 8 seq chunks, each [128x128]x[128x512] fp16 = 512 cycles), which is
the compute floor. Channel dim per core = 512 -> 4 groups of 128 partitions;
groups alternate between two sets of 4 PSUM banks (8 banks total), and each
type's consumers (sigmoid / copy / kv-mul+scan / final muls) fire as soon as
that type's 8-matmul accumulation completes, so PSUM recycling never stalls
the PE. Output is written transposed [512, S] and untransposed on the host.
"""

import numpy as np

import concourse.bass as bass
import concourse.bacc as bacc
import concourse.tile as tile
import concourse.mybir as mybir
from concourse.bass_utils import run_bass_kernel_spmd

B, S, D = 4, 4096, 1024
H = 512          # output channels per core (half of D)
P = 128
KT = D // P      # 8 contraction tiles
FD = 512         # seq columns per chunk (= PSUM bank capacity in fp32)
NJ = S // FD     # 8 seq chunks
NG = H // P      # 4 channel groups of 128
C = 4 * H        # 2048 projection columns per core (g,k,v,q per group)

f32 = mybir.dt.float32
f16 = mybir.dt.float16

# types within a group, in matmul order: gate first (longest consumer chain
# start), then k, v (kv-mul + scan), then q (final muls + store)
T_G, T_K, T_V, T_Q = 0, 1, 2, 3

_NC_CACHE = {}


def _build(with_bias: bool):
    nc = bacc.Bacc("TRN2", target_bir_lowering=False)

    # host supplies x^T and W^T pre-packed so one chunk (resp. weight group)
    # is contiguous per partition (8KB lines): descriptor generation on the
    # sync queue is ~4ns/descriptor serial, so a 1MB transfer must be 128
    # descriptors, not 1024 -- otherwise the PE stalls at startup
    xT_d = nc.dram_tensor("xT", [NJ, P, KT, FD], f16, kind="ExternalInput")
    wt_d = nc.dram_tensor("wt", [NG, P, KT, 4 * P], f16, kind="ExternalInput")
    if with_bias:
        bias_d = nc.dram_tensor("bias", [P, 4 * NG], f32, kind="ExternalInput")
    out_d = nc.dram_tensor("out", [H, S], f32, kind="ExternalOutput")

    sig = mybir.ActivationFunctionType.Sigmoid
    ident = mybir.ActivationFunctionType.Identity
    mult = mybir.AluOpType.mult
    add = mybir.AluOpType.add

    with tile.TileContext(nc) as tc:
        with (
            tc.tile_pool(name="consts", bufs=1) as consts,
            tc.tile_pool(name="xin", bufs=2) as xin,
            tc.tile_pool(name="work", bufs=1) as work,
            tc.tile_pool(name="scanp", bufs=2) as scanp,
            tc.tile_pool(name="outp", bufs=3) as outp,
            tc.tile_pool(name="psp", bufs=1, space="PSUM") as psp,
        ):
            GC = 4 * P  # 512 columns per group
            # ~8 dummy matmuls on uninitialized SBUF: keeps the PE busy for
            # the ~3.4us HAM activity window while the first DMAs land, so
            # the real matmul stream starts at 2.4GHz instead of 1.2GHz
            warm_w = consts.tile([P, P], f16, tag="warm_w")
            warm_x = consts.tile([P, FD], f16, tag="warm_x")
            nc.gpsimd.memset(warm_w[:], 0.0)
            nc.gpsimd.memset(warm_x[:], 0.0)
            warm_ps = psp.tile([P, FD], f32, tag="ps0_0", name="warm_ps")
            # >= 3.4us of sustained matmul activity (8 x 427ns cold) releases
            # the HAM clock gate (1.2 -> 2.4GHz), and the warm stream must
            # butt up against the first real matmul or the gate re-throttles
            for _ in range(8):
                nc.tensor.matmul(warm_ps[:], warm_w[:], warm_x[:],
                                 start=True, stop=True)

            # staged startup DMAs: a single dma's descriptors drain on ONE
            # queue (~50GB/s for 256KB), so (group0, chunk0) land as 128KB
            # per-kt pieces fanned out across several issue engines/queues;
            # the first real matmul only needs the kt0 pieces
            wtg_sb = [
                consts.tile([P, KT, GC], f16, tag=f"wtg{gi}", name=f"wtg{gi}")
                for gi in range(NG)
            ]
            xts0 = xin.tile([P, KT, FD], f16, tag="x", name="xts0")
            # staged startup: a DMA costs ~0.6us serial issue on its ring +
            # ~2.3us latency + size/100GB/s on its queue, and group0 consumes
            # a (wt, x) kt pair every ~0.86us once running -- so wt kt-singles
            # issue on the sync ring while x kt-singles issue in parallel on
            # the scalar ring, and group1's weights ride the (otherwise idle)
            # gpsimd ring so they land before group0 finishes
            for kt in range(KT):
                nc.sync.dma_start(wtg_sb[0][:, kt, :], wt_d[0, :, kt, :])
            for kt in range(KT):
                nc.scalar.dma_start(xts0[:, kt, :], xT_d[0, :, kt, :])
            nc.gpsimd.dma_start(wtg_sb[1][:, 0:KT // 2, :],
                                wt_d[1, :, 0:KT // 2, :])
            nc.gpsimd.dma_start(wtg_sb[1][:, KT // 2:KT, :],
                                wt_d[1, :, KT // 2:KT, :])
            ones_sb = consts.tile([P, FD], f32, tag="ones")
            nc.vector.memset(ones_sb[:], 1.0)
            if with_bias:
                bias_sb = consts.tile([P, 4 * NG], f32, tag="bias")
                nc.sync.dma_start(bias_sb[:], bias_d[:])

            prev_scan = {}
            for j in range(NJ):
                if j == 0:
                    xts = xts0
                else:
                    xts = xin.tile([P, KT, FD], f16, tag="x", name="xts")
                    nc.sync.dma_start(xts[:], xT_d[j])

                for i in range(NG):
                    par = i % 2
                    ps = [
                        psp.tile([P, FD], f32, tag=f"ps{t}_{par}",
                                 name=f"ps{t}_{par}")
                        for t in range(4)
                    ]
                    bcol = (
                        (lambda t: bias_sb[:, (i * 4 + t):(i * 4 + t) + 1])
                        if with_bias else None
                    )
                    last = (j == NJ - 1 and i == NG - 1)
                    g_sb = k_sb = kv_sb = sc = gs_sb = None
                    for t in range(4):
                        col0 = t * P
                        for kt in range(KT):
                            nc.tensor.matmul(
                                ps[t][:],
                                wtg_sb[i][:, kt, col0:col0 + P],
                                xts[:, kt, :],
                                start=(kt == 0),
                                stop=(kt == KT - 1),
                            )
                        # consumers fire as soon as this type's accumulation
                        # is complete, staggered across the group's matmuls
                        if t == T_G:
                            g_sb = work.tile([P, FD], f32, tag=f"g{par}")
                            nc.scalar.activation(
                                g_sb[:], ps[T_G][:], sig,
                                bias=bcol(T_G) if with_bias else 0.0,
                            )
                        elif t == T_K:
                            k_sb = work.tile([P, FD], f32, tag=f"k{par}")
                            if with_bias:
                                nc.scalar.activation(
                                    k_sb[:], ps[T_K][:], ident, bias=bcol(T_K)
                                )
                            else:
                                nc.scalar.copy(k_sb[:], ps[T_K][:])
                        elif t == T_V:
                            kv_sb = work.tile([P, FD], f32, tag=f"kv{par}")
                            if with_bias:
                                nc.vector.scalar_tensor_tensor(
                                    kv_sb[:], ps[T_V][:], bcol(T_V), k_sb[:],
                                    add, mult,
                                )
                            else:
                                nc.vector.tensor_mul(
                                    out=kv_sb[:], in0=k_sb[:], in1=ps[T_V][:]
                                )
                            sc = scanp.tile([P, FD], f32, tag=f"scan{i}")
                            init = 0.0 if j == 0 else prev_scan[i][:, FD - 1:FD]
                            # state = (ones * state) + kv ; out[t] = state
                            nc.vector.tensor_tensor_scan(
                                sc[:], ones_sb[:], kv_sb[:], init, mult, add
                            )
                            prev_scan[i] = sc
                            # gs = sigmoid(g) * kv_state, computed during the
                            # q matmuls so only ONE mul remains after they end
                            gs_sb = work.tile([P, FD], f32, tag=f"gs{par}")
                            nc.vector.tensor_mul(
                                out=gs_sb[:], in0=g_sb[:], in1=sc[:]
                            )
                        else:  # T_Q
                            ob = outp.tile([P, FD], f32, tag="ob")
                            if with_bias:
                                nc.vector.scalar_tensor_tensor(
                                    ob[:], ps[T_Q][:], bcol(T_Q), gs_sb[:],
                                    add, mult,
                                )
                            else:
                                nc.vector.tensor_mul(
                                    out=ob[:], in0=gs_sb[:], in1=ps[T_Q][:]
                                )
                            # a single-queue 256KB dma drains at ~50GB/s; the
                            # last chunk's stores gate the exit barrier (and
                            # ob-buffer reuse), so split them across queues
                            nc.sync.dma_start(
                                out_d[i * P:(i + 1) * P,
                                      j * FD:(j + 1) * FD],
                                ob[:],
                            )
                    if j == 0 and 2 <= i + 1 < NG:
                        # weight groups 2-3 stream in behind group i's compute
                        # (group1 went out on the gpsimd ring at startup)
                        nc.sync.dma_start(wtg_sb[i + 1][:], wt_d[i + 1])

    nc.compile()
    return nc


def _get_nc(with_bias: bool):
    if with_bias not in _NC_CACHE:
        _NC_CACHE[with_bias] = _build(with_bias)
    return _NC_CACHE[with_bias]


def _prep_in_maps(x, W_qkv, b_qkv, W_gate, b_gate, with_bias):
    x = np.asarray(x, dtype=np.float32)
    W_qkv = np.asarray(W_qkv, dtype=np.float32)
    W_gate = np.asarray(W_gate, dtype=np.float32)

    # x^T packed [chunk j, partition p, kt, col c] = x[j*FD+c, kt*128+p]
    # so one chunk's DMA is 8KB-contiguous per partition (128 descriptors)
    xTs = [
        np.ascontiguousarray(
            x[b].astype(np.float16).reshape(NJ, FD, KT, P).transpose(0, 3, 2, 1)
        )
        for b in range(B)
    ]

    # weight rows per (group, type): [g_i | k_i | v_i | q_i] blocks of 128
    wts, biases = [], []
    for h in range(2):
        blocks, bcols = [], []
        for i in range(NG):
            r0 = h * H + i * P
            rows = [
                W_gate[r0:r0 + P],
                W_qkv[D + r0:D + r0 + P],
                W_qkv[2 * D + r0:2 * D + r0 + P],
                W_qkv[r0:r0 + P],
            ]
            blocks.extend(rows)
            if with_bias:
                bq = np.asarray(b_qkv, dtype=np.float32)
                bg = np.asarray(b_gate, dtype=np.float32)
                bcols.extend([
                    bg[r0:r0 + P],
                    bq[D + r0:D + r0 + P],
                    bq[2 * D + r0:2 * D + r0 + P],
                    bq[r0:r0 + P],
                ])
        wt = np.concatenate(blocks, axis=0).T  # [1024, 2048]
        # packed [group gi, partition p, kt, col c] = wt[kt*128+p, gi*512+c]
        wts.append(np.ascontiguousarray(
            wt.astype(np.float16).reshape(KT, P, NG, 4 * P).transpose(2, 1, 0, 3)
        ))
        if with_bias:
            biases.append(np.stack(bcols, axis=1).astype(np.float32))  # [128,16]

    in_maps = []
    for core in range(8):
        b, h = core // 2, core % 2
        m = {"xT": xTs[b], "wt": wts[h]}
        if with_bias:
            m["bias"] = biases[h]
        in_maps.append(m)
    return in_maps


def run(x, W_qkv, b_qkv, W_gate, b_gate, trace=False, **run_kwargs):
    with_bias = bool(np.any(np.asarray(b_qkv)) or np.any(np.asarray(b_gate)))
    nc = _get_nc(with_bias)
    in_maps = _prep_in_maps(x, W_qkv, b_qkv, W_gate, b_gate, with_bias)
    res = run_bass_kernel_spmd(nc, in_maps, list(range(8)), trace=trace, **run_kwargs)
    out = np.empty((B, S, D), dtype=np.float32)
    for core in range(8):
        b, h = core // 2, core % 2
        out[b, :, h * H:(h + 1) * H] = res.results[core]["out"].T
    return out, res


def kernel(x, W_qkv, b_qkv, W_gate, b_gate):
    out, _ = run(x, W_qkv, b_qkv, W_gate, b_gate)
    return out


# revision 50
# speedup vs baseline: 1.0450x; 1.0450x over previous
"""Trainium2 Bass kernel for nn_LinearLatentKernel_84834194031187.

Computes, for x:[B,S,D], W_qkv:[3D,D], W_gate:[D,D] (fp32):
    qkv = x @ W_qkv.T + b_qkv ; q,k,v = split(qkv)
    kv_state = cumsum(k*v, axis=seq)
    out = q * kv_state * sigmoid(x @ W_gate.T + b_gate)

Sharding: 8 cores = (batch b in 0..3) x (channel half h in 0..1); each core
computes out[b, :, h*512:(h+1)*512].

Layout: everything is computed TRANSPOSED, [channel, seq], so that
  - the projection matmuls take host-pretransposed x^T [D,S] directly as the
    moving operand (no PE transposes at all): ps[c,s] = wt[d,c]^T @ x^T[d,s]
  - the cumsum along seq becomes a FREE-dim prefix scan on the Vector engine
    (tensor_tensor_scan, chained across 512-col chunks via initial=prev[:,-1:])
so the Tensor engine runs ONLY the 1024 projection matmuls (8 kt x 4 types x
4 groups x 8 seq chunks, each [128x128]x[128x512] fp16 = 512 cycles), which is
the compute floor. Channel dim per core = 512 -> 4 groups of 128 partitions;
groups alternate between two sets of 4 PSUM banks (8 banks total), and each
type's consumers (sigmoid / copy / kv-mul+scan / final muls) fire as soon as
that type's 8-matmul accumulation completes, so PSUM recycling never stalls
the PE. Output is written transposed [512, S] and untransposed on the host.
"""

import numpy as np

import concourse.bass as bass
import concourse.bacc as bacc
import concourse.tile as tile
import concourse.mybir as mybir
from concourse.bass_utils import run_bass_kernel_spmd

B, S, D = 4, 4096, 1024
H = 512          # output channels per core (half of D)
P = 128
KT = D // P      # 8 contraction tiles
FD = 512         # seq columns per chunk (= PSUM bank capacity in fp32)
NJ = S // FD     # 8 seq chunks
NG = H // P      # 4 channel groups of 128
C = 4 * H        # 2048 projection columns per core (g,k,v,q per group)

f32 = mybir.dt.float32
f16 = mybir.dt.float16

# types within a group, in matmul order: gate first (longest consumer chain
# start), then k, v (kv-mul + scan), then q (final muls + store)
T_G, T_K, T_V, T_Q = 0, 1, 2, 3

_NC_CACHE = {}


def _build(with_bias: bool):
    nc = bacc.Bacc("TRN2", target_bir_lowering=False)

    # host supplies x^T and W^T pre-packed so one chunk (resp. weight group)
    # is contiguous per partition (8KB lines): descriptor generation on the
    # sync queue is ~4ns/descriptor serial, so a 1MB transfer must be 128
    # descriptors, not 1024 -- otherwise the PE stalls at startup
    xT_d = nc.dram_tensor("xT", [NJ, P, KT, FD], f16, kind="ExternalInput")
    wt_d = nc.dram_tensor("wt", [NG, P, KT, 4 * P], f16, kind="ExternalInput")
    if with_bias:
        bias_d = nc.dram_tensor("bias", [P, 4 * NG], f32, kind="ExternalInput")
    out_d = nc.dram_tensor("out", [H, S], f32, kind="ExternalOutput")

    sig = mybir.ActivationFunctionType.Sigmoid
    ident = mybir.ActivationFunctionType.Identity
    mult = mybir.AluOpType.mult
    add = mybir.AluOpType.add

    with tile.TileContext(nc) as tc:
        with (
            tc.tile_pool(name="consts", bufs=1) as consts,
            tc.tile_pool(name="xin", bufs=2) as xin,
            tc.tile_pool(name="work", bufs=1) as work,
            tc.tile_pool(name="scanp", bufs=2) as scanp,
            tc.tile_pool(name="outp", bufs=6) as outp,
            tc.tile_pool(name="psp", bufs=1, space="PSUM") as psp,
        ):
            GC = 4 * P  # 512 columns per group
            # ~8 dummy matmuls on uninitialized SBUF: keeps the PE busy for
            # the ~3.4us HAM activity window while the first DMAs land, so
            # the real matmul stream starts at 2.4GHz instead of 1.2GHz
            warm_w = consts.tile([P, P], f16, tag="warm_w")
            warm_x = consts.tile([P, FD], f16, tag="warm_x")
            nc.gpsimd.memset(warm_w[:], 0.0)
            nc.gpsimd.memset(warm_x[:], 0.0)
            warm_ps = psp.tile([P, FD], f32, tag="ps0_0", name="warm_ps")
            # >= 3.4us of sustained matmul activity (8 x 427ns cold) releases
            # the HAM clock gate (1.2 -> 2.4GHz), and the warm stream must
            # butt up against the first real matmul or the gate re-throttles
            for _ in range(8):
                nc.tensor.matmul(warm_ps[:], warm_w[:], warm_x[:],
                                 start=True, stop=True)

            # staged startup DMAs: a single dma's descriptors drain on ONE
            # queue (~50GB/s for 256KB), so (group0, chunk0) land as 128KB
            # per-kt pieces fanned out across several issue engines/queues;
            # the first real matmul only needs the kt0 pieces
            wtg_sb = [
                consts.tile([P, KT, GC], f16, tag=f"wtg{gi}", name=f"wtg{gi}")
                for gi in range(NG)
            ]
            xts0 = xin.tile([P, KT, FD], f16, tag="x", name="xts0")
            # staged startup DMAs: (group0, chunk0) land in kt-group pieces
            # so the first real matmuls start as soon as kt0-1 are in.
            # NOTE: finer piecing starves the ~11-deep DMA semaphore pool
            # (the issue ring head-of-line blocks on semaphore recycling),
            # measured 253us vs 242us -- keep exactly six pieces here.
            for lo, hi in ((0, 2), (2, 4), (4, 8)):
                nc.sync.dma_start(wtg_sb[0][:, lo:hi, :], wt_d[0, :, lo:hi, :])
                nc.sync.dma_start(xts0[:, lo:hi, :], xT_d[0, :, lo:hi, :])
            ones_sb = consts.tile([P, FD], f32, tag="ones")
            nc.vector.memset(ones_sb[:], 1.0)
            if with_bias:
                bias_sb = consts.tile([P, 4 * NG], f32, tag="bias")
                nc.sync.dma_start(bias_sb[:], bias_d[:])

            prev_scan = {}
            for j in range(NJ):
                if j == 0:
                    xts = xts0
                else:
                    xts = xin.tile([P, KT, FD], f16, tag="x", name="xts")
                    nc.sync.dma_start(xts[:], xT_d[j])

                for i in range(NG):
                    par = i % 2
                    ps = [
                        psp.tile([P, FD], f32, tag=f"ps{t}_{par}",
                                 name=f"ps{t}_{par}")
                        for t in range(4)
                    ]
                    bcol = (
                        (lambda t: bias_sb[:, (i * 4 + t):(i * 4 + t) + 1])
                        if with_bias else None
                    )
                    last = (j == NJ - 1 and i == NG - 1)
                    g_sb = k_sb = kv_sb = sc = gs_sb = None
                    for t in range(4):
                        col0 = t * P
                        for kt in range(KT):
                            nc.tensor.matmul(
                                ps[t][:],
                                wtg_sb[i][:, kt, col0:col0 + P],
                                xts[:, kt, :],
                                start=(kt == 0),
                                stop=(kt == KT - 1),
                            )
                        # consumers fire as soon as this type's accumulation
                        # is complete, staggered across the group's matmuls
                        if t == T_G:
                            g_sb = work.tile([P, FD], f32, tag=f"g{par}")
                            nc.scalar.activation(
                                g_sb[:], ps[T_G][:], sig,
                                bias=bcol(T_G) if with_bias else 0.0,
                            )
                        elif t == T_K:
                            k_sb = work.tile([P, FD], f32, tag=f"k{par}")
                            if with_bias:
                                nc.scalar.activation(
                                    k_sb[:], ps[T_K][:], ident, bias=bcol(T_K)
                                )
                            else:
                                nc.scalar.copy(k_sb[:], ps[T_K][:])
                        elif t == T_V:
                            kv_sb = work.tile([P, FD], f32, tag=f"kv{par}")
                            if with_bias:
                                nc.vector.scalar_tensor_tensor(
                                    kv_sb[:], ps[T_V][:], bcol(T_V), k_sb[:],
                                    add, mult,
                                )
                            else:
                                nc.vector.tensor_mul(
                                    out=kv_sb[:], in0=k_sb[:], in1=ps[T_V][:]
                                )
                            sc = scanp.tile([P, FD], f32, tag=f"scan{i}")
                            init = 0.0 if j == 0 else prev_scan[i][:, FD - 1:FD]
                            # state = (ones * state) + kv ; out[t] = state
                            nc.vector.tensor_tensor_scan(
                                sc[:], ones_sb[:], kv_sb[:], init, mult, add
                            )
                            prev_scan[i] = sc
                            # gs = sigmoid(g) * kv_state, computed during the
                            # q matmuls so only ONE mul remains after they end
                            gs_sb = work.tile([P, FD], f32, tag=f"gs{par}")
                            nc.vector.tensor_mul(
                                out=gs_sb[:], in0=g_sb[:], in1=sc[:]
                            )
                        else:  # T_Q
                            ob = outp.tile([P, FD], f32, tag="ob")
                            if with_bias:
                                nc.vector.scalar_tensor_tensor(
                                    ob[:], ps[T_Q][:], bcol(T_Q), gs_sb[:],
                                    add, mult,
                                )
                            else:
                                nc.vector.tensor_mul(
                                    out=ob[:], in0=gs_sb[:], in1=ps[T_Q][:]
                                )
                            # a single-queue 256KB dma drains at ~50GB/s; the
                            # last chunk's stores gate the exit barrier (and
                            # ob-buffer reuse), so split them across queues
                            nc.sync.dma_start(
                                out_d[i * P:(i + 1) * P,
                                      j * FD:(j + 1) * FD],
                                ob[:],
                            )
                    if j == 0 and i + 1 < NG:
                        # weight groups 1-3 stream in behind group i's compute
                        nc.sync.dma_start(wtg_sb[i + 1][:], wt_d[i + 1])

    nc.compile()
    return nc


def _get_nc(with_bias: bool):
    if with_bias not in _NC_CACHE:
        _NC_CACHE[with_bias] = _build(with_bias)
    return _NC_CACHE[with_bias]


def _prep_in_maps(x, W_qkv, b_qkv, W_gate, b_gate, with_bias):
    x = np.asarray(x, dtype=np.float32)
    W_qkv = np.asarray(W_qkv, dtype=np.float32)
    W_gate = np.asarray(W_gate, dtype=np.float32)

    # x^T packed [chunk j, partition p, kt, col c] = x[j*FD+c, kt*128+p]
    # so one chunk's DMA is 8KB-contiguous per partition (128 descriptors)
    xTs = [
        np.ascontiguousarray(
            x[b].astype(np.float16).reshape(NJ, FD, KT, P).transpose(0, 3, 2, 1)
        )
        for b in range(B)
    ]

    # weight rows per (group, type): [g_i | k_i | v_i | q_i] blocks of 128
    wts, biases = [], []
    for h in range(2):
        blocks, bcols = [], []
        for i in range(NG):
            r0 = h * H + i * P
            rows = [
                W_gate[r0:r0 + P],
                W_qkv[D + r0:D + r0 + P],
                W_qkv[2 * D + r0:2 * D + r0 + P],
                W_qkv[r0:r0 + P],
            ]
            blocks.extend(rows)
            if with_bias:
                bq = np.asarray(b_qkv, dtype=np.float32)
                bg = np.asarray(b_gate, dtype=np.float32)
                bcols.extend([
                    bg[r0:r0 + P],
                    bq[D + r0:D + r0 + P],
                    bq[2 * D + r0:2 * D + r0 + P],
                    bq[r0:r0 + P],
                ])
        wt = np.concatenate(blocks, axis=0).T  # [1024, 2048]
        # packed [group gi, partition p, kt, col c] = wt[kt*128+p, gi*512+c]
        wts.append(np.ascontiguousarray(
            wt.astype(np.float16).reshape(KT, P, NG, 4 * P).transpose(2, 1, 0, 3)
        ))
        if with_bias:
            biases.append(np.stack(bcols, axis=1).astype(np.float32))  # [128,16]

    in_maps = []
    for core in range(8):
        b, h = core // 2, core % 2
        m = {"xT": xTs[b], "wt": wts[h]}
        if with_bias:
            m["bias"] = biases[h]
        in_maps.append(m)
    return in_maps


def run(x, W_qkv, b_qkv, W_gate, b_gate, trace=False, **run_kwargs):
    with_bias = bool(np.any(np.asarray(b_qkv)) or np.any(np.asarray(b_gate)))
    nc = _get_nc(with_bias)
    in_maps = _prep_in_maps(x, W_qkv, b_qkv, W_gate, b_gate, with_bias)
    res = run_bass_kernel_spmd(nc, in_maps, list(range(8)), trace=trace, **run_kwargs)
    out = np.empty((B, S, D), dtype=np.float32)
    for core in range(8):
        b, h = core // 2, core % 2
        out[b, :, h * H:(h + 1) * H] = res.results[core]["out"].T
    return out, res


def kernel(x, W_qkv, b_qkv, W_gate, b_gate):
    out, _ = run(x, W_qkv, b_qkv, W_gate, b_gate)
    return out
